# revision 11
# baseline (speedup 1.0000x reference)
"""Two-layer LSTM (B=128, T=256, I=512, H=1024) on 8 trn2 NeuronCores.

Strategy: tensor-parallel over the 4H gate dimension. Core k owns hidden
units [k*128,(k+1)*128) of both layers (512 gate rows each, column order
i|f|o|g). Matmuls run with batch(=128) as the stationary/M dim and the
512 gate columns streaming as N, in float32r (FP22) at full PE rate.
The recurrence all-gathers the transposed h-slices of both layers once
per step (single 8-core AllGather of 128KB per rank, via HBM bounce).
Layer-0's input matmul (x @ W_ih0^T + b) for all T is precomputed into
DRAM up front. Raw bass (no Tile), 5-engine pipeline, fully unrolled.
"""

import sys

for p in ("/opt/trn_rl_repo", "/opt/pypackages"):
    if p not in sys.path:
        sys.path.insert(0, p)

import numpy as np

import concourse.bass as bass
import concourse.mybir as mybir
from concourse.bass_utils import run_bass_kernel_spmd

P = 128
B = 128
I_DIM = 512
H = 1024
NC = 8
GS = 512          # gate rows per core (4 gates x 128 hidden)
KH = H // P       # 8 k-tiles over hidden
KI = I_DIM // P   # 4 k-tiles over input
F32 = mybir.dt.float32
F32R = mybir.dt.float32r


def build(T, dbg_y0=False):
    """Build the unrolled SPMD program for T timesteps."""
    nc = bass.Bass()

    # ---- kernel I/O (per core) ----
    xTd = nc.dram_tensor("xT", [I_DIM, T * B], F32, kind="ExternalInput")
    whh0d = nc.dram_tensor("whh0t", [H, GS], F32, kind="ExternalInput")
    wih1d = nc.dram_tensor("wih1t", [H, GS], F32, kind="ExternalInput")
    whh1d = nc.dram_tensor("whh1t", [H, GS], F32, kind="ExternalInput")
    wih0d = nc.dram_tensor("wih0t", [I_DIM, GS], F32, kind="ExternalInput")
    b0d = nc.dram_tensor("b0", [P, GS], F32, kind="ExternalInput")
    b1d = nc.dram_tensor("b1", [P, GS], F32, kind="ExternalInput")
    identd = nc.dram_tensor("ident", [P, P], F32, kind="ExternalInput")
    yd = nc.dram_tensor("y", [B, T, P], F32, kind="ExternalOutput")

    # ---- internal DRAM ----
    gx0d = nc.dram_tensor("gx0", [T * B, GS], F32)  # t-major: row t*B+b
    cc_in = nc.dram_tensor("cc_in", [2, 2, P, P], F32)
    cc_out = nc.dram_tensor("cc_out", [2, NC, 2, P, P], F32, addr_space="Shared")
    gx0d_r = gx0d.rearrange("(t p) f -> t p f", p=P)

    import contextlib

    ctx = contextlib.ExitStack()
    with ctx:
        sb = lambda shape, name, dt=F32: ctx.enter_context(nc.sbuf_tensor(name, shape, dt))
        ps = lambda name: ctx.enter_context(nc.psum_tensor(name, [P, GS], F32))

        w_hh0 = sb([P, KH, GS], "w_hh0", F32R)
        w_ih1 = sb([P, KH, GS], "w_ih1", F32R)
        w_hh1 = sb([P, KH, GS], "w_hh1", F32R)
        w_ih0 = sb([P, KI, GS], "w_ih0", F32R)
        b0s = sb([P, GS], "b0s", F32R)
        b1s = sb([P, GS], "b1s", F32R)
        ident_s = sb([P, P], "ident_s")
        identr_s = sb([P, P], "identr_s", F32R)
        xt_t = sb([P, 4, KI, P], "xt_t", F32R)  # 4-deep ring of xT tiles
        pc_s = sb([P, 2, GS], "pc_s")           # precompute result staging
        gx_t = sb([P, 2, GS], "gx_t", F32R)     # per-step layer0 x-gates
        h0f = sb([P, 2, KH, P], "h0f", F32R)    # gathered h0^T (hid x batch)
        h1f = sb([P, 2, KH, P], "h1f", F32R)
        hts = sb([P, 2, 2, P], "hts")           # send staging [slot0=h0T, slot1=h1T]
        sig0 = sb([P, 2, 384], "sig0")
        sig1 = sb([P, 2, 384], "sig1")
        tg0 = sb([P, 2, P], "tg0")
        tg1 = sb([P, 2, P], "tg1")
        tc0 = sb([P, 2, P], "tc0")
        tc1 = sb([P, 2, P], "tc1")
        tmp0 = sb([P, 2, P], "tmp0")
        tmp1 = sb([P, 2, P], "tmp1")
        h0s = sb([P, 2, P], "h0s")
        h1s = sb([P, 2, P], "h1s")
        c0 = sb([P, P], "c0")
        c1 = sb([P, P], "c1")

        g0p = [ps("g0p_a"), ps("g0p_b")]        # also precompute psum
        g1p = [ps("g1p_a"), ps("g1p_b")]
        trp0 = [ps("trp0_a"), ps("trp0_b")]     # full bank; use [:, :128]
        trp1 = [ps("trp1_a"), ps("trp1_b")]

        sem_names = [
            "S_LOAD", "S_INIT", "S_PPE", "S_PCP", "S_PGX",
            "S_CC", "S_GO0", "S_GO1", "S_GIN",
            "S_PE0", "S_PE1", "S_A0", "S_A1", "S_C0", "S_C1",
            "S_TC0", "S_TC1", "S_H0", "S_H1", "S_TR0", "S_TR1",
            "S_CP0", "S_CP1", "S_Y",
        ]
        S = {n: ctx.enter_context(nc.semaphore(n)) for n in sem_names}
        S["S_PXs"] = [ctx.enter_context(nc.semaphore(f"S_PX{q}")) for q in range(4)]
        S["S_GXp"] = [ctx.enter_context(nc.semaphore(f"S_GX{q}")) for q in range(2)]

        xTd_r = xTd.rearrange("(o p) (t f) -> p o t f", p=P, f=P)
        whh0_r = whh0d.rearrange("(o p) f -> p o f", p=P)
        wih1_r = wih1d.rearrange("(o p) f -> p o f", p=P)
        whh1_r = whh1d.rearrange("(o p) f -> p o f", p=P)
        wih0_r = wih0d.rearrange("(o p) f -> p o f", p=P)

        with nc.Block() as block:

            @block.sync
            def _(sp):
                # prologue loads
                loads = [
                    (w_hh0[:], whh0_r.bitcast(F32R)), (w_ih1[:], wih1_r.bitcast(F32R)),
                    (w_hh1[:], whh1_r.bitcast(F32R)), (w_ih0[:], wih0_r.bitcast(F32R)),
                    (b0s[:], b0d[:].bitcast(F32R)), (b1s[:], b1d[:].bitcast(F32R)),
                    (ident_s[:], identd[:]),
                    (identr_s[:], identd[:].bitcast(F32R)),
                ]
                for dst, src in loads:
                    sp.dma_start(dst, src).then_inc(S["S_LOAD"], 16)
                # precompute: stream xT tiles in, gx0 results out
                for j in range(T):
                    if j >= 4:
                        sp.wait_ge(S["S_PPE"], j - 3)
                    sp.dma_start(xt_t[:, j % 4], xTd_r[:, :, j, :].bitcast(F32R)).then_inc(S["S_PXs"][j % 4], 16)
                    sp.wait_ge(S["S_PCP"], j + 1)
                    sp.dma_start(gx0d_r[j], pc_s[:, j % 2]).then_inc(S["S_PGX"], 16)
                # main loop
                sp.wait_ge(S["S_INIT"], 3)
                for i in range(T + 1):
                    if i <= T - 1:
                        if i >= 2:
                            sp.wait_ge(S["S_PE0"], i - 1)   # gx_t parity WAR
                        sp.wait_ge(S["S_PGX"], 16 * (i + 1))
                        sp.dma_start(gx_t[:, i % 2], gx0d_r[i].bitcast(F32R)).then_inc(S["S_GXp"][i % 2], 16)
                    if i >= 1:
                        sp.wait_ge(S["S_CC"], i)
                        sp.dma_start(
                            h0f[:, i % 2], cc_out[i % 2, :, 0].rearrange("k p f -> p k f").bitcast(F32R)
                        ).then_inc(S["S_GO0"], 16)
                        sp.dma_start(
                            h1f[:, i % 2], cc_out[i % 2, :, 1].rearrange("k p f -> p k f").bitcast(F32R)
                        ).then_inc(S["S_GO1"], 16)
                    if i <= T - 1:
                        sp.wait_ge(S["S_CP0"], i + 1)
                        if i >= 1:
                            sp.wait_ge(S["S_CP1"], i)
                        sp.dma_start(
                            cc_in[(i + 1) % 2].rearrange("s p f -> p s f"), hts[:, i % 2]
                        ).then_inc(S["S_GIN"], 16)
                    if dbg_y0:
                        if i <= T - 1:
                            sp.wait_ge(S["S_H0"], i + 1)
                            sp.dma_start(yd[:, i, :], h0s[:, i % 2]).then_inc(S["S_Y"], 16)
                    elif i >= 1:
                        sp.wait_ge(S["S_H1"], i)
                        sp.dma_start(yd[:, i - 1, :], h1s[:, i % 2]).then_inc(S["S_Y"], 16)
                sp.wait_ge(S["S_Y"], 16 * T)

            @block.gpsimd
            def _(gp):
                for i in range(1, T + 1):
                    gp.wait_ge(S["S_GIN"], 16 * i)
                    if i >= 2:
                        gp.wait_ge(S["S_GO0"], 16 * (i - 1))
                        gp.wait_ge(S["S_GO1"], 16 * (i - 1))
                    gp.collective_compute(
                        "AllGather",
                        mybir.AluOpType.bypass,
                        replica_groups=[list(range(NC))],
                        ins=[cc_in[i % 2]],
                        outs=[cc_out[i % 2]],
                    ).then_inc(S["S_CC"], 1)

            @block.tensor
            def _(pe):
                pe.wait_ge(S["S_LOAD"], 16 * 8)
                # precompute gx0 = x @ W_ih0^T + b0
                for j in range(T):
                    pe.wait_ge(S["S_PXs"][j % 4], 16 * (j // 4 + 1))
                    if j >= 2:
                        pe.wait_ge(S["S_PCP"], j - 1)   # psum parity WAR
                    pcp = g0p[j % 2]
                    pe.matmul(pcp[:], identr_s[:], b0s[:], start=True, stop=False)
                    for k in range(KI):
                        mm = pe.matmul(
                            pcp[:], xt_t[:, j % 4, k, :], w_ih0[:, k, :],
                            start=False, stop=(k == KI - 1),
                        )
                        if k == KI - 1:
                            mm.then_inc(S["S_PPE"], 1)
                pe.wait_ge(S["S_PCP"], T)   # precompute fully drained from psum
                # main loop
                for i in range(T + 1):
                    par = i % 2
                    if i >= 1:
                        # g1(i-1) = b1 + Wih1 . h0(i-1) + Whh1 . h1(i-2)
                        if i >= 3:
                            pe.wait_ge(S["S_A1"], i - 2)   # ACT done reading this psum bank
                        pe.matmul(g1p[par][:], identr_s[:], b1s[:], start=True, stop=False)
                        pe.wait_ge(S["S_GO0"], 16 * i)
                        for k in range(KH):
                            pe.matmul(
                                g1p[par][:], h0f[:, par, k, :], w_ih1[:, k, :],
                                start=False, stop=False,
                            )
                        pe.wait_ge(S["S_GO1"], 16 * i)
                        for k in range(KH):
                            mm = pe.matmul(
                                g1p[par][:], h1f[:, par, k, :], w_hh1[:, k, :],
                                start=False, stop=(k == KH - 1),
                            )
                            if k == KH - 1:
                                mm.then_inc(S["S_PE1"], 1)
                    if i <= T - 1:
                        # g0(i) = gx0(i) + Whh0 . h0(i-1)   (h0(-1) = 0)
                        pe.wait_ge(S["S_GXp"][i % 2], 16 * (i // 2 + 1))
                        if i >= 2:
                            pe.wait_ge(S["S_A0"], i - 1)   # ACT done reading this psum bank
                        if i == 0:
                            pe.matmul(
                                g0p[par][:], identr_s[:], gx_t[:, par], start=True, stop=True
                            ).then_inc(S["S_PE0"], 1)
                        else:
                            pe.matmul(g0p[par][:], identr_s[:], gx_t[:, par], start=True, stop=False)
                            for k in range(KH):
                                mm = pe.matmul(
                                    g0p[par][:], h0f[:, par, k, :], w_hh0[:, k, :],
                                    start=False, stop=(k == KH - 1),
                                )
                                if k == KH - 1:
                                    mm.then_inc(S["S_PE0"], 1)
                    if 1 <= i <= T - 1:
                        pe.wait_ge(S["S_H1"], i)
                        pe.transpose(trp1[par][:, :P], h1s[:, par], ident_s[:]).then_inc(S["S_TR1"], 1)
                    if i <= T - 1:
                        pe.wait_ge(S["S_H0"], i + 1)
                        pe.transpose(trp0[par][:, :P], h0s[:, par], ident_s[:]).then_inc(S["S_TR0"], 1)

            @block.scalar
            def _(act):
                Sig = mybir.ActivationFunctionType.Sigmoid
                Tanh = mybir.ActivationFunctionType.Tanh
                for i in range(T + 1):
                    par = i % 2
                    if i >= 1:
                        act.wait_ge(S["S_PE1"], i)
                        act.activation(sig1[:, par], g1p[par][:, 0:384], Sig)
                        act.activation(tg1[:, par], g1p[par][:, 384:512], Tanh).then_inc(S["S_A1"], 1)
                    if i <= T - 1:
                        act.wait_ge(S["S_PE0"], i + 1)
                        act.activation(sig0[:, par], g0p[par][:, 0:384], Sig)
                        act.activation(tg0[:, par], g0p[par][:, 384:512], Tanh).then_inc(S["S_A0"], 1)
                    if i >= 1:
                        act.wait_ge(S["S_C1"], i)
                        act.activation(tc1[:, par], c1[:], Tanh).then_inc(S["S_TC1"], 1)
                    if i <= T - 1:
                        act.wait_ge(S["S_C0"], i + 1)
                        act.activation(tc0[:, par], c0[:], Tanh).then_inc(S["S_TC0"], 1)

            @block.vector
            def _(dv):
                for t_ap in (c0[:], c1[:], hts[:, 0, 1, :]):
                    dv.memset(t_ap, 0.0).then_inc(S["S_INIT"], 1)
                for j in range(T):
                    dv.wait_ge(S["S_PPE"], j + 1)
                    if j >= 2:
                        dv.wait_ge(S["S_PGX"], 16 * j)   # pc_s parity WAR (all issued)
                    dv.tensor_copy(pc_s[:, j % 2], g0p[j % 2][:]).then_inc(S["S_PCP"], 1)
                for i in range(T + 1):
                    par = i % 2
                    if i >= 1:
                        dv.wait_ge(S["S_A1"], i)
                        dv.tensor_mul(c1[:], sig1[:, par, 128:256], c1[:])
                        dv.tensor_mul(tmp1[:, par], sig1[:, par, 0:128], tg1[:, par])
                        dv.tensor_add(c1[:], c1[:], tmp1[:, par]).then_inc(S["S_C1"], 1)
                    if i <= T - 1:
                        dv.wait_ge(S["S_A0"], i + 1)
                        dv.tensor_mul(c0[:], sig0[:, par, 128:256], c0[:])
                        dv.tensor_mul(tmp0[:, par], sig0[:, par, 0:128], tg0[:, par])
                        dv.tensor_add(c0[:], c0[:], tmp0[:, par]).then_inc(S["S_C0"], 1)
                    if i >= 1:
                        dv.wait_ge(S["S_TC1"], i)
                        if i >= 2:
                            dv.wait_ge(S["S_Y"], 16 * (i - 1))
                        dv.tensor_mul(h1s[:, par], sig1[:, par, 256:384], tc1[:, par]).then_inc(S["S_H1"], 1)
                    if i <= T - 1:
                        dv.wait_ge(S["S_TC0"], i + 1)
                        dv.tensor_mul(h0s[:, par], sig0[:, par, 256:384], tc0[:, par]).then_inc(S["S_H0"], 1)
                    if 1 <= i <= T - 1:
                        dv.wait_ge(S["S_TR1"], i)
                        if i >= 2:
                            dv.wait_ge(S["S_GIN"], 16 * (i - 1))
                        dv.tensor_copy(hts[:, par, 1, :], trp1[par][:, :P]).then_inc(S["S_CP1"], 1)
                    if i <= T - 1:
                        dv.wait_ge(S["S_TR0"], i + 1)
                        dv.tensor_copy(hts[:, par, 0, :], trp0[par][:, :P]).then_inc(S["S_CP0"], 1)

    return nc


def shard_inputs(x, W_ih0, W_hh0, b_ih0, b_hh0, W_ih1, W_hh1, b_ih1, b_hh1):
    """Host-side prep: per-core pre-transposed weight slices + shared xT."""
    Bx, T, _ = x.shape
    xT = np.ascontiguousarray(
        x.astype(np.float32).transpose(2, 1, 0).reshape(I_DIM, T * Bx)
    )
    ident = np.eye(P, dtype=np.float32)
    b0 = (b_ih0 + b_hh0).astype(np.float32)
    b1 = (b_ih1 + b_hh1).astype(np.float32)
    in_maps = []
    for k in range(NC):
        hk = np.arange(k * P, (k + 1) * P)
        rows = np.concatenate([0 * H + hk, 1 * H + hk, 3 * H + hk, 2 * H + hk])  # i|f|o|g
        in_maps.append({
            "xT": xT,
            "whh0t": np.ascontiguousarray(W_hh0[rows].T.astype(np.float32)),
            "wih1t": np.ascontiguousarray(W_ih1[rows].T.astype(np.float32)),
            "whh1t": np.ascontiguousarray(W_hh1[rows].T.astype(np.float32)),
            "wih0t": np.ascontiguousarray(W_ih0[rows].T.astype(np.float32)),
            "b0": np.ascontiguousarray(np.tile(b0[rows][None, :], (P, 1))),
            "b1": np.ascontiguousarray(np.tile(b1[rows][None, :], (P, 1))),
            "ident": ident,
        })
    return in_maps


def run(x, W_ih0, W_hh0, b_ih0, b_hh0, W_ih1, W_hh1, b_ih1, b_hh1, trace=False, dbg_y0=False):
    x = np.asarray(x, dtype=np.float32)
    args = [np.asarray(a, dtype=np.float32) for a in
            (W_ih0, W_hh0, b_ih0, b_hh0, W_ih1, W_hh1, b_ih1, b_hh1)]
    Bx, T, _ = x.shape
    nc = build(T, dbg_y0=dbg_y0)
    in_maps = shard_inputs(x, *args)
    res = run_bass_kernel_spmd(nc, in_maps, core_ids=list(range(NC)), trace=trace)
    y = np.empty((Bx, T, H), dtype=np.float32)
    for k in range(NC):
        y[:, :, k * P:(k + 1) * P] = res.results[k]["y"]
    return y, res


def kernel(x, W_ih0, W_hh0, b_ih0, b_hh0, W_ih1, W_hh1, b_ih1, b_hh1):
    y, _ = run(x, W_ih0, W_hh0, b_ih0, b_hh0, W_ih1, W_hh1, b_ih1, b_hh1)
    return y
